# revision 10
# baseline (speedup 1.0000x reference)
"""BsPINN forward MLP on 8 TRN2 NeuronCores (Bass/Tile), data-parallel over rows.

Network (per reference):
  h = 2*(X-lb)/(ub-lb)-1          [N,3]   (folded into W0/b0 on host)
  h = sin(h @ W0 + b0)            [N,1024]
  h = sin(h @ W1 + b1)            [N,1024] dense
  h = sin(h @ (W2*m2) + b2)       [N,1024] block-diag 2x(512x512)
  h = sin(h @ (W3*m3) + b3)       [N,1024] block-diag 4x(256x256)
  out = h @ W4 + b4               [N,1]

Design: activations kept feature-major on chip (hT: [features->partitions,
rows->free]); out_chunkT = W_chunk.T @ hT via nc.tensor.matmul (lhsT=W is
already the stationary operand). Matmuls run in float32r (fp32 rounded to
~12-bit mantissa, 1 cycle/row vs 4 for fp32). The block-diagonal masks are
exploited by only multiplying in-block K-chunks. Sin runs on the scalar
engine (accurate to ~1e-7 for |x|<3.3; preactivations here stay < 0.4).
"""
import os
import numpy as np

import concourse.bass as bass
import concourse.tile as tile
from concourse import bacc, mybir
from concourse.bass_utils import run_bass_kernel_spmd

N_CORES = 8
N_FULL = 131072
R = N_FULL // N_CORES          # 16384 rows per core
NT = 512                       # matmul moving free dim (one PSUM bank, fp32)
RT = R // NT                   # 32 row tiles per core
NCH = 8                        # feature chunks (1024 / 128)

F32 = mybir.dt.float32
F32R = mybir.dt.float32r
SIN = mybir.ActivationFunctionType.Sin
IDENT = mybir.ActivationFunctionType.Identity

LAST_RESULTS = None
_PROGRAM = None


def _build_program(rt_count=RT, n_cores=N_CORES):
    nc = bacc.Bacc("TRN2", target_bir_lowering=False, debug=False,
                   num_devices=n_cores)

    xt_d = nc.dram_tensor("xt", [4, R], F32, kind="ExternalInput").ap()
    w0_d = nc.dram_tensor("w0", [4, 1024], F32, kind="ExternalInput").ap()
    w1_d = nc.dram_tensor("w1", [8, 128, 1024], F32, kind="ExternalInput").ap()
    w2_d = nc.dram_tensor("w2", [8, 128, 512], F32, kind="ExternalInput").ap()
    w3_d = nc.dram_tensor("w3", [8, 128, 256], F32, kind="ExternalInput").ap()
    w4_d = nc.dram_tensor("w4", [128, 16], F32, kind="ExternalInput").ap()
    bias_d = nc.dram_tensor("bias", [128, 32], F32, kind="ExternalInput").ap()
    b4_d = nc.dram_tensor("b4", [1, 1], F32, kind="ExternalInput").ap()
    o_d = nc.dram_tensor("o", [RT, NT], F32, kind="ExternalOutput").ap()

    with tile.TileContext(nc) as tc:
        with (
            tc.tile_pool(name="const", bufs=1) as cpool,
            tc.tile_pool(name="stage", bufs=2) as stpool,
            tc.tile_pool(name="hbuf", bufs=2) as hpool,
            tc.tile_pool(name="xio", bufs=2) as xpool,
            tc.tile_pool(name="psum", bufs=8, space="PSUM") as ppool,
        ):
            # ---- one-time weight/bias setup (DMA f32, round to f32r) ----
            bt = cpool.tile([128, 32], F32, name="bt", tag="bt")
            nc.sync.dma_start(out=bt[:], in_=bias_d[:])
            b4t = cpool.tile([1, 1], F32, name="b4t", tag="b4t")
            nc.sync.dma_start(out=b4t[:], in_=b4_d[:])

            stg = stpool.tile([128, 1024], F32, name="stg", tag="stg")
            nc.sync.dma_start(out=stg[:4, :], in_=w0_d[:])
            w0r = cpool.tile([4, 1024], F32R, name="w0r", tag="w0r")
            nc.vector.tensor_copy(w0r[:], stg[:4, :])

            stg = stpool.tile([128, 1024], F32, name="stg", tag="stg")
            nc.sync.dma_start(out=stg[:, :16], in_=w4_d[:])
            w4r = cpool.tile([128, 16], F32R, name="w4r", tag="w4r")
            nc.vector.tensor_copy(w4r[:], stg[:, :16])

            w1r, w2r, w3r = [], [], []
            for kc in range(NCH):
                stg = stpool.tile([128, 1024], F32, name="stg", tag="stg")
                nc.sync.dma_start(out=stg[:], in_=w1_d[kc])
                t1 = cpool.tile([128, 1024], F32R, name=f"w1r{kc}",
                                tag=f"w1r{kc}")
                nc.vector.tensor_copy(t1[:], stg[:])
                w1r.append(t1)
            for kc in range(NCH):
                stg = stpool.tile([128, 1024], F32, name="stg", tag="stg")
                nc.sync.dma_start(out=stg[:, :512], in_=w2_d[kc])
                t2 = cpool.tile([128, 512], F32R, name=f"w2r{kc}",
                                tag=f"w2r{kc}")
                nc.vector.tensor_copy(t2[:], stg[:, :512])
                w2r.append(t2)
            for kc in range(NCH):
                stg = stpool.tile([128, 1024], F32, name="stg", tag="stg")
                nc.sync.dma_start(out=stg[:, :256], in_=w3_d[kc])
                t3 = cpool.tile([128, 256], F32R, name=f"w3r{kc}",
                                tag=f"w3r{kc}")
                nc.vector.tensor_copy(t3[:], stg[:, :256])
                w3r.append(t3)

            # ---- row-tile loop ----
            for rt in range(rt_count):
                cs = rt * NT
                xs = xpool.tile([4, NT], F32, name="xs", tag="xs")
                nc.sync.dma_start(out=xs[:], in_=xt_d[:, cs:cs + NT])
                xr = xpool.tile([4, NT], F32R, name="xr", tag="xr")
                nc.vector.tensor_copy(xr[:], xs[:])

                # L0: K=4 (padded from 3)
                h1 = []
                for mc in range(NCH):
                    pt = ppool.tile([128, NT], F32, name="pt", tag="pt")
                    nc.tensor.matmul(pt[:], w0r[:, 128 * mc:128 * (mc + 1)],
                                     xr[:], start=True, stop=True)
                    h = hpool.tile([128, NT], F32R, name=f"h1_{mc}",
                                   tag=f"h1_{mc}")
                    nc.scalar.activation(h[:], pt[:], SIN,
                                         bias=bt[:, mc:mc + 1])
                    h1.append(h)

                # L1: dense, K=1024
                h2 = []
                for mc in range(NCH):
                    pt = ppool.tile([128, NT], F32, name="pt", tag="pt")
                    ks = [(mc + j) % NCH for j in range(NCH)]
                    for j, kc in enumerate(ks):
                        nc.tensor.matmul(pt[:],
                                         w1r[kc][:, 128 * mc:128 * (mc + 1)],
                                         h1[kc][:],
                                         start=(j == 0), stop=(j == NCH - 1))
                    h = hpool.tile([128, NT], F32R, name=f"h2_{mc}",
                                   tag=f"h2_{mc}")
                    nc.scalar.activation(h[:], pt[:], SIN,
                                         bias=bt[:, 8 + mc:9 + mc])
                    h2.append(h)

                # L2: block-diag 2 x (512x512): out chunk mc <- in chunks of
                # block b = mc//4
                h3 = []
                for mc in range(NCH):
                    b = mc // 4
                    co = (mc % 4) * 128
                    pt = ppool.tile([128, NT], F32, name="pt", tag="pt")
                    ks = [(mc + j) % 4 for j in range(4)]
                    for j, kcl in enumerate(ks):
                        nc.tensor.matmul(pt[:],
                                         w2r[4 * b + kcl][:, co:co + 128],
                                         h2[4 * b + kcl][:],
                                         start=(j == 0), stop=(j == 3))
                    h = hpool.tile([128, NT], F32R, name=f"h3_{mc}",
                                   tag=f"h3_{mc}", bufs=1)
                    nc.scalar.activation(h[:], pt[:], SIN,
                                         bias=bt[:, 16 + mc:17 + mc])
                    h3.append(h)

                # L3: block-diag 4 x (256x256): out chunk mc <- block mc//2
                h4 = []
                for mc in range(NCH):
                    bi = mc // 2
                    co = (mc % 2) * 128
                    pt = ppool.tile([128, NT], F32, name="pt", tag="pt")
                    ks = [(mc + j) % 2 for j in range(2)]
                    for j, kcl in enumerate(ks):
                        nc.tensor.matmul(pt[:],
                                         w3r[2 * bi + kcl][:, co:co + 128],
                                         h3[2 * bi + kcl][:],
                                         start=(j == 0), stop=(j == 1))
                    h = hpool.tile([128, NT], F32R, name=f"h4_{mc}",
                                   tag=f"h4_{mc}", bufs=1)
                    nc.scalar.activation(h[:], pt[:], SIN,
                                         bias=bt[:, 24 + mc:25 + mc])
                    h4.append(h)

                # L4: out = h4 @ W4 + b4, M padded to 2
                pt = ppool.tile([128, NT], F32, name="pt", tag="pt")
                for kc in range(NCH):
                    nc.tensor.matmul(pt[0:2, :], w4r[:, 2 * kc:2 * kc + 2],
                                     h4[kc][:],
                                     start=(kc == 0), stop=(kc == NCH - 1))
                ot = xpool.tile([1, NT], F32, name="ot", tag="ot")
                nc.scalar.activation(ot[:], pt[0:1, :], IDENT, bias=b4t[:])
                nc.gpsimd.dma_start(out=o_d[rt:rt + 1, :], in_=ot[0:1, :])

    nc.compile()
    return nc


def _get_program():
    global _PROGRAM
    if _PROGRAM is None:
        _PROGRAM = _build_program()
    return _PROGRAM


def kernel(X, lb_X, ub_X, W0, b0, W1, b1, W2, b2, W3, b3, W4, b4):
    X = np.asarray(X, np.float32)
    lb = np.asarray(lb_X, np.float64)
    ub = np.asarray(ub_X, np.float64)
    W0 = np.asarray(W0, np.float64)
    b0 = np.asarray(b0, np.float64)

    # fold input normalization h = X*s + t into W0/b0:
    #   sin((X*s+t)@W0 + b0) = sin(X@(s[:,None]*W0) + (t@W0 + b0))
    s = 2.0 / (ub - lb)
    t = -2.0 * lb / (ub - lb) - 1.0
    W0p = np.zeros((4, 1024), np.float32)
    W0p[:3] = (s[:, None] * W0).astype(np.float32)
    b0p = (b0 + t @ W0).astype(np.float32).reshape(1024)

    W1 = np.asarray(W1, np.float32)
    W2 = np.asarray(W2, np.float32)
    W3 = np.asarray(W3, np.float32)
    W4 = np.asarray(W4, np.float32)

    w1h = np.ascontiguousarray(W1.reshape(8, 128, 1024))
    # W2: 2 blocks of 512x512 -> [4b+kcl] = W2[512b+128kcl:+128, 512b:+512]
    w2h = np.zeros((8, 128, 512), np.float32)
    for b in range(2):
        for kcl in range(4):
            w2h[4 * b + kcl] = W2[512 * b + 128 * kcl:512 * b + 128 * (kcl + 1),
                                  512 * b:512 * (b + 1)]
    # W3: 4 blocks of 256x256 -> [2bi+kcl] = W3[256bi+128kcl:+128, 256bi:+256]
    w3h = np.zeros((8, 128, 256), np.float32)
    for bi in range(4):
        for kcl in range(2):
            w3h[2 * bi + kcl] = W3[256 * bi + 128 * kcl:256 * bi + 128 * (kcl + 1),
                                   256 * bi:256 * (bi + 1)]
    # W4 [1024,1] -> [128,16]: [p, 2kc] = W4[128kc+p, 0]; odd cols zero pad
    w4h = np.zeros((128, 16), np.float32)
    w4h[:, 0::2] = W4.reshape(8, 128).T
    # biases [1,1024] -> [128, 8] chunk-major columns; 4 layers side by side
    bh = np.zeros((128, 32), np.float32)
    for i, bb in enumerate([b0p, np.asarray(b1, np.float32).reshape(1024),
                            np.asarray(b2, np.float32).reshape(1024),
                            np.asarray(b3, np.float32).reshape(1024)]):
        bh[:, 8 * i:8 * (i + 1)] = bb.reshape(8, 128).T
    b4h = np.asarray(b4, np.float32).reshape(1, 1)

    nc = _get_program()

    in_maps = []
    for c in range(N_CORES):
        xt = np.zeros((4, R), np.float32)
        xt[:3] = X[c * R:(c + 1) * R].T
        in_maps.append({
            "xt": xt, "w0": W0p, "w1": w1h, "w2": w2h, "w3": w3h,
            "w4": w4h, "bias": bh, "b4": b4h,
        })

    trace = bool(int(os.environ.get("KERNEL_TRACE", "0")))
    res = run_bass_kernel_spmd(nc, in_maps, list(range(N_CORES)), trace=trace)
    global LAST_RESULTS
    LAST_RESULTS = res

    out = np.concatenate([res.results[c]["o"].reshape(R) for c in range(N_CORES)])
    return out.reshape(N_FULL, 1).astype(np.float32)
